# revision 16
# baseline (speedup 1.0000x reference)
"""Trainium2 Bass kernel: per-pixel 5x5 kernel application (KPN-style).

    out[b,c,y,x] = sum_{i,j} softmax(kernels[b,:,y,x])[i*5+j]
                   * zpad(data)[b,c,y+i,x+j]          (i,j in 0..4, r=2)

Sharding (8 NeuronCores, pure data parallel, no collectives):
    core = (b, H-half): 4 batches x 2 row-bands of 360 rows.
    Halo rows come from overlapping host-side slices of the full input.

Per-core algorithm (tiles live in "data space": 124 partitions =
120 output rows + 2 halo rows each side):
    - kernels ship as int8 (k*SCALE); ACT does et = exp(kt/SCALE) straight
      from int8, one instruction per di-group (FD = 5*1280).
    - product planes Q_t = E_t * D (bf16): the x-shift dj is a free-dim
      offset (a second parity copy of the data, built by one ACT copy,
      keeps operands 4B-aligned); the y-shift di is folded into the kernel
      DMA (rows loaded shifted by -di) and undone on the PE by a stationary
      shift matrix S_di[k,m] = [k == m+di].  DVE computes SAME-PARITY TAP
      PAIRS (dj 0&2, 1&3) in one 4-dim op each - the dj-axis stride of 2
      bf16 elements stays 4B-aligned so the 2x perf mode survives, and the
      ~0.4us/op drain+init overhead is paid half as often.  The dj=4 taps
      run on GPSIMD concurrently (tensor_tensor never enters the 2-port
      perf mode that locks the Q7s out of SBUF).
    - PE accumulates the 25 tap planes (3 channels) and the 25 exp planes
      (softmax denominator) into 4 PSUM banks per 512-col x-chunk,
      ping-ponged across two 4-bank PSUM tiles so the PE never waits on
      the per-chunk finalize.
    - finalize per chunk: DVE reciprocal_approx_fast of the sumexp bank
      (the exact `reciprocal` is an 8 cyc/elem iterative divide); ACT
      converts it to bf16 and stages the numerator banks to SBUF bf16
      (keeps the final multiply in DVE 2x mode); DVE multiplies; output is
      stored as bf16 and upcast on the host.

DMA notes: the two HWDGE rings share the same 4 SDMA engines (~94 GB/s
combined) while SWDGE fans over all 16 (~150+ GB/s), so all loads ride
SWDGE.  The SDMA engines drain concurrent DMAs round-robin with no
priority, so issue ORDER is the only latency control: the Q7 stream
interleaves descriptor generation with the product ops - kt0+data first,
then the rest of the current tile, and the next row-tile's loads one and
two x-chunks later.  Output stores ride the otherwise-idle sync ring
(HWDGE), dependency-gated so they never flood the startup.

kernel(**inputs) takes the FULL inputs and returns the FULL output.
"""

import numpy as np
import ml_dtypes

B, C, H, W, KW = 4, 3, 720, 1280, 5
NCORES = 8
HS = H // 2            # 360 output rows per shard
RT = 120               # output rows per row-tile
NRT = HS // RT         # 3 row-tiles
HALO = 2
DP = RT + 2 * HALO     # 124 partitions (data space)
WP = 1288              # padded data width: 2 left + 1280 + 6 right
KROWPAD = 4            # zero rows around each kernel shard (top+bottom)
KH = HS + 2 * KROWPAD  # 368
XCH = [(0, 512), (512, 512), (1024, 256)]

KERN_INT8 = True       # ship kernels as int8 (quarters HBM traffic vs f32)
KSCALE = 23.0          # int8 code = round(k * KSCALE); |k| <= 5.42 -> <125
OUT_BF16 = True        # store output as bf16, upcast on host

_CACHE = {}


def _build_program():
    import concourse.bacc as bacc
    import concourse.mybir as mybir
    from concourse.bass import AP
    from concourse import tile

    f32 = mybir.dt.float32
    bf16 = mybir.dt.bfloat16
    kdt = mybir.dt.int8 if KERN_INT8 else bf16
    odt = bf16 if OUT_BF16 else f32

    nc = bacc.Bacc(
        "TRN2",
        target_bir_lowering=False,
        debug=False,
        enable_asserts=False,
        num_devices=NCORES,
    )
    d_data = nc.dram_tensor("data", [HS + 2 * HALO, C, WP], bf16, kind="ExternalInput")
    d_kern = nc.dram_tensor("kern", [KH, KW * KW, W], kdt, kind="ExternalInput")
    d_out = nc.dram_tensor("out", [HS, C, W], odt, kind="ExternalOutput")

    # Shift matrices S_di[k, m] = 1 iff k == m + di.  Host layout
    # [DP, KW, RT] so the load is 124 contiguous 1200B descriptors.
    s_np = np.zeros((DP, KW, RT), dtype=ml_dtypes.bfloat16)
    for di in range(KW):
        for m in range(RT):
            s_np[m + di, di, m] = 1.0
    d_s = nc.inline_tensor(np.ascontiguousarray(s_np), "smat")

    KROW = KW * KW * W  # element stride between rows of d_kern
    escale = float(1.0 / KSCALE) if KERN_INT8 else 1.0

    with tile.TileContext(nc) as tc:
        with tc.tile_pool(name="const", bufs=1) as cpool, \
             tc.tile_pool(name="dbf", bufs=2) as dbfpool, \
             tc.tile_pool(name="kt", bufs=5) as kpool, \
             tc.tile_pool(name="et", bufs=6) as epool, \
             tc.tile_pool(name="qt", bufs=3) as qpool, \
             tc.tile_pool(name="qg", bufs=2) as qgpool, \
             tc.tile_pool(name="fin", bufs=2) as fpool, \
             tc.tile_pool(name="ps", bufs=2, space="PSUM") as ppool:

            s_sb = cpool.tile([DP, KW, RT], bf16)
            nc.gpsimd.dma_start(out=s_sb[:], in_=d_s.ap())

            kt_tiles = {}
            dbf_tiles = {}

            def load_kt(rt, di):
                if rt >= NRT:
                    return
                kt = kpool.tile([DP, KW, W], kdt, tag="kt")
                kt_tiles[(rt, di)] = kt
                off = (KROWPAD + rt * RT - di) * KROW + di * KW * W
                nc.gpsimd.dma_start(
                    out=kt[:], in_=AP(d_kern, off, [[KROW, DP], [1, KW * W]])
                )

            def load_dbf(rt):
                if rt >= NRT:
                    return
                dbf = dbfpool.tile([DP, 2, C, WP], bf16, tag="dbf")
                dbf_tiles[rt] = dbf
                nc.gpsimd.dma_start(
                    out=dbf[:, 0], in_=d_data.ap()[rt * RT:rt * RT + DP]
                )

            # first row-tile's loads, urgency-ordered; kt3/kt4 are
            # issued later from inside the first x-chunk's Q7 stream so
            # the round-robin SDMA drain gives kt0+data a head start.
            load_kt(0, 0)
            load_dbf(0)
            load_kt(0, 1)
            load_kt(0, 2)

            def emit_exp(rt, di, ets):
                kt = kt_tiles.pop((rt, di))
                et = epool.tile([DP, KW, W], bf16, tag="et")
                nc.scalar.activation(
                    et[:].rearrange("p a b -> p (a b)"),
                    kt[:].rearrange("p a b -> p (a b)"),
                    mybir.ActivationFunctionType.Exp,
                    scale=escale,
                )
                ets[di] = et

            for rt in range(NRT):
                dbf = dbf_tiles[rt]
                CWP = C * WP

                ets = {}
                for di in range(3 if rt == 0 else KW):
                    emit_exp(rt, di, ets)
                    if di == 0:
                        # parity copy: dbf1[x] = dbf0[x+1] keeps odd-dj
                        # product operands 4B-aligned.
                        nc.scalar.copy(
                            dbf[:, 1, :, 0:WP - 1], dbf[:, 0, :, 1:WP]
                        )

                ost = fpool.tile([RT, C, W], odt, tag="ost")

                for xi, (xc, xcw) in enumerate(XCH):
                    # PSUM banks: 0..2 = channel accumulators, 3 = sumexp
                    pacc = ppool.tile([RT, 4, 512], f32, tag="pacc")

                    for di in range(KW):
                        et = ets[di]
                        lhs = s_sb[:, di, :]
                        first = di == 0
                        last = di == KW - 1
                        for dj in range(KW):
                            nc.tensor.matmul(
                                out=pacc[:, 3, 0:xcw],
                                lhsT=lhs,
                                rhs=et[:, dj, xc:xc + xcw],
                                start=first and dj == 0,
                                stop=last and dj == KW - 1,
                            )
                        # same-parity tap groups: triple (0,2,4) on DVE,
                        # pair (1,3) on DVE for di 0/4, on GPSIMD for the
                        # middle di (concurrent engines, shared SBUF).
                        for gj, djs in enumerate(((0, 2, 4), (1, 3))):
                            ng = len(djs)
                            par = djs[0] % 2
                            pair_eng = nc.vector if di in (0, 4) else nc.gpsimd
                            eng = nc.vector if gj == 0 else pair_eng
                            if gj == 0:
                                qt = qpool.tile([DP, 3, C, 512], bf16, tag="qt")
                            else:
                                qt = qgpool.tile([DP, 2, C, 512], bf16, tag="qg")
                            base = par * CWP + xc + djs[0] - par
                            dsrc = AP(
                                dbf[:].tensor, dbf[:].offset + base,
                                [[2 * CWP, DP], [2, ng], [WP, C], [1, xcw]],
                            )
                            esrc = (
                                et[:, djs[0]:djs[-1] + 1:2, xc:xc + xcw]
                                .unsqueeze(2)
                                .broadcast_to([DP, ng, C, xcw])
                            )
                            eng.tensor_tensor(
                                qt[:, 0:ng, :, 0:xcw], esrc, dsrc,
                                mybir.AluOpType.mult,
                            )
                            for pi in range(ng):
                                dj = djs[pi]
                                for c in range(C):
                                    nc.tensor.matmul(
                                        out=pacc[:, c, 0:xcw],
                                        lhsT=lhs,
                                        rhs=qt[:, pi, c, 0:xcw],
                                        start=first and dj == 0,
                                        stop=last and dj == 3,
                                    )
                        if rt == 0 and xc == 0 and di == 0:
                            load_kt(0, 3)
                            emit_exp(0, 3, ets)
                        if rt == 0 and xc == 0 and di == 1:
                            load_kt(0, 4)
                            emit_exp(0, 4, ets)

                    # stagger the next row-tile's loads in the Q7 stream:
                    # after chunk 0 -> kt0 + data; after chunk 1 -> kt1..4.
                    if xi == 0:
                        load_kt(rt + 1, 0)
                        load_dbf(rt + 1)
                    elif xi == 1:
                        for di in range(1, KW):
                            load_kt(rt + 1, di)

                    # finalize this x-chunk
                    rsf = fpool.tile([RT, 512], f32, tag="rsf")
                    rsb = fpool.tile([RT, 512], bf16, tag="rsb")
                    nst = fpool.tile([RT, C, 512], bf16, tag="nst")
                    nc.vector.reciprocal_approx_fast(
                        rsf[:, 0:xcw], pacc[:, 3, 0:xcw]
                    )
                    nc.scalar.copy(rsb[:, 0:xcw], rsf[:, 0:xcw])
                    nc.scalar.copy(nst[:, :, 0:xcw], pacc[:, 0:3, 0:xcw])
                    rbc = rsb[:, 0:xcw].unsqueeze(1).broadcast_to([RT, C, xcw])
                    nc.vector.tensor_tensor(
                        ost[:, :, xc:xc + xcw], nst[:, :, 0:xcw], rbc,
                        mybir.AluOpType.mult,
                    )

                # store this row-tile on the otherwise-idle sync ring
                nc.sync.dma_start(out=d_out.ap()[rt * RT:rt * RT + RT], in_=ost[:])

    nc.compile()
    return nc


def get_program():
    if "nc" not in _CACHE:
        _CACHE["nc"] = _build_program()
    return _CACHE["nc"]


def make_shards(data: np.ndarray, kernels: np.ndarray):
    """Full inputs -> per-core input maps (with halo + zero padding)."""
    data = np.asarray(data, dtype=np.float32)
    kernels = np.asarray(kernels, dtype=np.float32)
    dpad = np.zeros((B, H + 2 * HALO, C, WP), dtype=ml_dtypes.bfloat16)
    dpad[:, HALO:HALO + H, :, 2:2 + W] = (
        data.transpose(0, 2, 1, 3).astype(ml_dtypes.bfloat16)
    )
    if KERN_INT8:
        kq = np.clip(np.round(kernels * KSCALE), -127, 127).astype(np.int8)
        kdt = np.int8
    else:
        kq = kernels.astype(ml_dtypes.bfloat16)
        kdt = ml_dtypes.bfloat16
    in_maps = []
    for core in range(NCORES):
        b, hh = divmod(core, 2)
        r0 = hh * HS
        dsh = np.ascontiguousarray(dpad[b, r0:r0 + HS + 2 * HALO])
        ksh = np.zeros((KH, KW * KW, W), dtype=kdt)
        ksh[KROWPAD:KROWPAD + HS] = kq[b, :, r0:r0 + HS, :].transpose(1, 0, 2)
        in_maps.append({"data": dsh, "kern": ksh})
    return in_maps


def assemble(results) -> np.ndarray:
    out = np.empty((B, C, H, W), dtype=np.float32)
    for core in range(NCORES):
        b, hh = divmod(core, 2)
        out[b, :, hh * HS:(hh + 1) * HS, :] = (
            results[core]["out"].astype(np.float32).transpose(1, 0, 2)
        )
    return out


def kernel(data: np.ndarray, kernels: np.ndarray) -> np.ndarray:
    from concourse.bass_utils import run_bass_kernel_spmd

    nc = get_program()
    in_maps = make_shards(data, kernels)
    res = run_bass_kernel_spmd(nc, in_maps, list(range(NCORES)))
    return assemble(res.results)


if __name__ == "__main__":
    get_program()
    print("program built OK")


# revision 17
# speedup vs baseline: 1.1130x; 1.1130x over previous
"""Trainium2 Bass kernel: per-pixel 5x5 kernel application (KPN-style).

    out[b,c,y,x] = sum_{i,j} softmax(kernels[b,:,y,x])[i*5+j]
                   * zpad(data)[b,c,y+i,x+j]          (i,j in 0..4, r=2)

Sharding (8 NeuronCores, pure data parallel, no collectives):
    core = (b, H-half): 4 batches x 2 row-bands of 360 rows.
    Halo rows come from overlapping host-side slices of the full input.

Per-core algorithm (tiles live in "data space": 124 partitions =
120 output rows + 2 halo rows each side):
    - kernels ship as int8 (k*SCALE); ACT does et = exp(kt/SCALE) straight
      from int8, one instruction per di-group (FD = 5*1280).
    - product planes Q_t = E_t * D (bf16): the x-shift dj is a free-dim
      offset (a second parity copy of the data, built by one ACT copy,
      keeps operands 4B-aligned); the y-shift di is folded into the kernel
      DMA (rows loaded shifted by -di) and undone on the PE by a stationary
      shift matrix S_di[k,m] = [k == m+di].  DVE computes SAME-PARITY TAP
      PAIRS (dj 0&2, 1&3) in one 4-dim op each - the dj-axis stride of 2
      bf16 elements stays 4B-aligned so the 2x perf mode survives, and the
      ~0.4us/op drain+init overhead is paid half as often.  The dj=4 taps
      run on GPSIMD concurrently (tensor_tensor never enters the 2-port
      perf mode that locks the Q7s out of SBUF).
    - PE accumulates the 25 tap planes (3 channels) and the 25 exp planes
      (softmax denominator) into 4 PSUM banks per 512-col x-chunk,
      ping-ponged across two 4-bank PSUM tiles so the PE never waits on
      the per-chunk finalize.
    - finalize per chunk: DVE reciprocal_approx_fast of the sumexp bank
      (the exact `reciprocal` is an 8 cyc/elem iterative divide); ACT
      converts it to bf16 and stages the numerator banks to SBUF bf16
      (keeps the final multiply in DVE 2x mode); DVE multiplies; output is
      stored as bf16 and upcast on the host.

DMA notes: the two HWDGE rings share the same 4 SDMA engines (~94 GB/s
combined) while SWDGE fans over all 16 (~150+ GB/s), so all loads ride
SWDGE.  The SDMA engines drain concurrent DMAs round-robin with no
priority, so issue ORDER is the only latency control: the Q7 stream
interleaves descriptor generation with the product ops - kt0+data first,
then the rest of the current tile, and the next row-tile's loads one and
two x-chunks later.  Output stores ride the otherwise-idle sync ring
(HWDGE), dependency-gated so they never flood the startup.

kernel(**inputs) takes the FULL inputs and returns the FULL output.
"""

import numpy as np
import ml_dtypes

B, C, H, W, KW = 4, 3, 720, 1280, 5
NCORES = 8
HS = H // 2            # 360 output rows per shard
RT = 120               # output rows per row-tile
NRT = HS // RT         # 3 row-tiles
HALO = 2
DP = RT + 2 * HALO     # 124 partitions (data space)
WP = 1288              # padded data width: 2 left + 1280 + 6 right
KROWPAD = 4            # zero rows around each kernel shard (top+bottom)
KH = HS + 2 * KROWPAD  # 368
XCH = [(0, 512), (512, 512), (1024, 256)]

KERN_INT8 = True       # ship kernels as int8 (quarters HBM traffic vs f32)
KSCALE = 23.0          # int8 code = round(k * KSCALE); |k| <= 5.42 -> <125
OUT_BF16 = True        # store output as bf16, upcast on host

_CACHE = {}


def _build_program():
    import concourse.bacc as bacc
    import concourse.mybir as mybir
    from concourse.bass import AP
    from concourse import tile

    f32 = mybir.dt.float32
    bf16 = mybir.dt.bfloat16
    kdt = mybir.dt.int8 if KERN_INT8 else bf16
    odt = bf16 if OUT_BF16 else f32

    nc = bacc.Bacc(
        "TRN2",
        target_bir_lowering=False,
        debug=False,
        enable_asserts=False,
        num_devices=NCORES,
    )
    d_data = nc.dram_tensor("data", [HS + 2 * HALO, C, WP], bf16, kind="ExternalInput")
    d_kern = nc.dram_tensor("kern", [KH, KW * KW, W], kdt, kind="ExternalInput")
    d_out = nc.dram_tensor("out", [HS, C, W], odt, kind="ExternalOutput")

    # Shift matrices S_di[k, m] = 1 iff k == m + di.  Host layout
    # [DP, KW, RT] so the load is 124 contiguous 1200B descriptors.
    s_np = np.zeros((DP, KW, RT), dtype=ml_dtypes.bfloat16)
    for di in range(KW):
        for m in range(RT):
            s_np[m + di, di, m] = 1.0
    d_s = nc.inline_tensor(np.ascontiguousarray(s_np), "smat")

    KROW = KW * KW * W  # element stride between rows of d_kern
    escale = float(1.0 / KSCALE) if KERN_INT8 else 1.0

    with tile.TileContext(nc) as tc:
        with tc.tile_pool(name="const", bufs=1) as cpool, \
             tc.tile_pool(name="dbf", bufs=2) as dbfpool, \
             tc.tile_pool(name="kt", bufs=5) as kpool, \
             tc.tile_pool(name="et", bufs=6) as epool, \
             tc.tile_pool(name="qt", bufs=3) as qpool, \
             tc.tile_pool(name="qg", bufs=3) as qgpool, \
             tc.tile_pool(name="fin", bufs=2) as fpool, \
             tc.tile_pool(name="ps", bufs=2, space="PSUM") as ppool:

            s_sb = cpool.tile([DP, KW, RT], bf16)
            nc.gpsimd.dma_start(out=s_sb[:], in_=d_s.ap())

            kt_tiles = {}
            dbf_tiles = {}

            def load_kt(rt, di):
                if rt >= NRT:
                    return
                kt = kpool.tile([DP, KW, W], kdt, tag="kt")
                kt_tiles[(rt, di)] = kt
                off = (KROWPAD + rt * RT - di) * KROW + di * KW * W
                nc.gpsimd.dma_start(
                    out=kt[:], in_=AP(d_kern, off, [[KROW, DP], [1, KW * W]])
                )

            def load_dbf(rt):
                if rt >= NRT:
                    return
                dbf = dbfpool.tile([DP, 2, C, WP], bf16, tag="dbf")
                dbf_tiles[rt] = dbf
                nc.gpsimd.dma_start(
                    out=dbf[:, 0], in_=d_data.ap()[rt * RT:rt * RT + DP]
                )

            # first row-tile's loads, urgency-ordered; kt3/kt4 are
            # issued later from inside the first x-chunk's Q7 stream so
            # the round-robin SDMA drain gives kt0+data a head start.
            load_kt(0, 0)
            load_dbf(0)
            load_kt(0, 1)
            load_kt(0, 2)

            def emit_exp(rt, di, ets):
                kt = kt_tiles.pop((rt, di))
                et = epool.tile([DP, KW, W], bf16, tag="et")
                nc.scalar.activation(
                    et[:].rearrange("p a b -> p (a b)"),
                    kt[:].rearrange("p a b -> p (a b)"),
                    mybir.ActivationFunctionType.Exp,
                    scale=escale,
                )
                ets[di] = et

            for rt in range(NRT):
                dbf = dbf_tiles[rt]
                CWP = C * WP

                ets = {}
                for di in range(3 if rt == 0 else KW):
                    emit_exp(rt, di, ets)
                    if di == 0:
                        # parity copy: dbf1[x] = dbf0[x+1] keeps odd-dj
                        # product operands 4B-aligned.
                        nc.scalar.copy(
                            dbf[:, 1, :, 0:WP - 1], dbf[:, 0, :, 1:WP]
                        )

                ost = fpool.tile([RT, C, W], odt, tag="ost")

                for xi, (xc, xcw) in enumerate(XCH):
                    # PSUM banks: 0..2 = channel accumulators, 3 = sumexp
                    pacc = ppool.tile([RT, 4, 512], f32, tag="pacc")

                    for di in range(KW):
                        et = ets[di]
                        lhs = s_sb[:, di, :]
                        first = di == 0
                        last = di == KW - 1
                        for dj in range(KW):
                            nc.tensor.matmul(
                                out=pacc[:, 3, 0:xcw],
                                lhsT=lhs,
                                rhs=et[:, dj, xc:xc + xcw],
                                start=first and dj == 0,
                                stop=last and dj == KW - 1,
                            )
                        # same-parity tap pairs (0,2) and (1,3) on DVE:
                        # the dj-axis stride of 2 bf16 elements stays
                        # 4B-aligned so the 2x perf mode survives and the
                        # ~0.4us/op drain+init overhead is paid half as
                        # often.  The dj=4 taps run on GPSIMD concurrently.
                        for pj, (d0, d1) in enumerate(((0, 2), (1, 3))):
                            qt = qpool.tile([DP, 2, C, 512], bf16, tag="qt")
                            par = d0 % 2
                            base = par * CWP + xc + d0 - par
                            dsrc = AP(
                                dbf[:].tensor, dbf[:].offset + base,
                                [[2 * CWP, DP], [2, 2], [WP, C], [1, xcw]],
                            )
                            esrc = (
                                et[:, d0:d1 + 1:2, xc:xc + xcw]
                                .unsqueeze(2)
                                .broadcast_to([DP, 2, C, xcw])
                            )
                            nc.vector.tensor_tensor(
                                qt[:, :, :, 0:xcw], esrc, dsrc,
                                mybir.AluOpType.mult,
                            )
                            for pi in range(2):
                                dj = (d0, d1)[pi]
                                for c in range(C):
                                    nc.tensor.matmul(
                                        out=pacc[:, c, 0:xcw],
                                        lhsT=lhs,
                                        rhs=qt[:, pi, c, 0:xcw],
                                        start=first and dj == 0 and pj == 0,
                                        stop=False,
                                    )
                        # dj = 4 tap on GPSIMD
                        qg = qgpool.tile([DP, C, 512], bf16, tag="qg")
                        dsrc = dbf[:, 0, :, xc + 4:xc + 4 + xcw]
                        esrc = (
                            et[:, 4, xc:xc + xcw]
                            .unsqueeze(1)
                            .broadcast_to([DP, C, xcw])
                        )
                        nc.gpsimd.tensor_tensor(
                            qg[:, :, 0:xcw], esrc, dsrc, mybir.AluOpType.mult,
                        )
                        for c in range(C):
                            nc.tensor.matmul(
                                out=pacc[:, c, 0:xcw],
                                lhsT=lhs,
                                rhs=qg[:, c, 0:xcw],
                                start=False,
                                stop=last,
                            )
                        if rt == 0 and xc == 0 and di == 0:
                            load_kt(0, 3)
                            emit_exp(0, 3, ets)
                        if rt == 0 and xc == 0 and di == 1:
                            load_kt(0, 4)
                            emit_exp(0, 4, ets)

                    # stagger the next row-tile's loads in the Q7 stream:
                    # after chunk 0 -> kt0 + data; after chunk 1 -> kt1..4.
                    if xi == 0:
                        load_kt(rt + 1, 0)
                        load_dbf(rt + 1)
                    elif xi == 1:
                        for di in range(1, KW):
                            load_kt(rt + 1, di)

                    # finalize this x-chunk
                    rsf = fpool.tile([RT, 512], f32, tag="rsf")
                    rsb = fpool.tile([RT, 512], bf16, tag="rsb")
                    nst = fpool.tile([RT, C, 512], bf16, tag="nst")
                    nc.vector.reciprocal_approx_fast(
                        rsf[:, 0:xcw], pacc[:, 3, 0:xcw]
                    )
                    nc.scalar.copy(rsb[:, 0:xcw], rsf[:, 0:xcw])
                    nc.scalar.copy(nst[:, :, 0:xcw], pacc[:, 0:3, 0:xcw])
                    rbc = rsb[:, 0:xcw].unsqueeze(1).broadcast_to([RT, C, xcw])
                    nc.vector.tensor_tensor(
                        ost[:, :, xc:xc + xcw], nst[:, :, 0:xcw], rbc,
                        mybir.AluOpType.mult,
                    )

                # store this row-tile on the otherwise-idle sync ring
                nc.sync.dma_start(out=d_out.ap()[rt * RT:rt * RT + RT], in_=ost[:])

    nc.compile()
    return nc


def get_program():
    if "nc" not in _CACHE:
        _CACHE["nc"] = _build_program()
    return _CACHE["nc"]


def make_shards(data: np.ndarray, kernels: np.ndarray):
    """Full inputs -> per-core input maps (with halo + zero padding)."""
    data = np.asarray(data, dtype=np.float32)
    kernels = np.asarray(kernels, dtype=np.float32)
    dpad = np.zeros((B, H + 2 * HALO, C, WP), dtype=ml_dtypes.bfloat16)
    dpad[:, HALO:HALO + H, :, 2:2 + W] = (
        data.transpose(0, 2, 1, 3).astype(ml_dtypes.bfloat16)
    )
    if KERN_INT8:
        kq = np.clip(np.round(kernels * KSCALE), -127, 127).astype(np.int8)
        kdt = np.int8
    else:
        kq = kernels.astype(ml_dtypes.bfloat16)
        kdt = ml_dtypes.bfloat16
    in_maps = []
    for core in range(NCORES):
        b, hh = divmod(core, 2)
        r0 = hh * HS
        dsh = np.ascontiguousarray(dpad[b, r0:r0 + HS + 2 * HALO])
        ksh = np.zeros((KH, KW * KW, W), dtype=kdt)
        ksh[KROWPAD:KROWPAD + HS] = kq[b, :, r0:r0 + HS, :].transpose(1, 0, 2)
        in_maps.append({"data": dsh, "kern": ksh})
    return in_maps


def assemble(results) -> np.ndarray:
    out = np.empty((B, C, H, W), dtype=np.float32)
    for core in range(NCORES):
        b, hh = divmod(core, 2)
        out[b, :, hh * HS:(hh + 1) * HS, :] = (
            results[core]["out"].astype(np.float32).transpose(1, 0, 2)
        )
    return out


def kernel(data: np.ndarray, kernels: np.ndarray) -> np.ndarray:
    from concourse.bass_utils import run_bass_kernel_spmd

    nc = get_program()
    in_maps = make_shards(data, kernels)
    res = run_bass_kernel_spmd(nc, in_maps, list(range(NCORES)))
    return assemble(res.results)


if __name__ == "__main__":
    get_program()
    print("program built OK")
